# revision 5
# baseline (speedup 1.0000x reference)
"""Trainium2 Bass kernel: bidirectional ligand<->protein cross-attention.

Strategy: batch ids are sorted, so the lig/pro attention mask is block-
diagonal over the 32 batches. Each batch is an independent attention
problem (both directions). We give each of the 8 cores 4 batches, padded
to uniform slots (lig rows -> 128, pro rows -> CPC, a multiple of 128),
so one SPMD program serves all cores. Padding is neutralized with an
additive -1e5 mask folded into the score matmul as a rank-1 (K=1)
accumulation. The distance bias exp(-cdist/10) is computed on-device via
a K=5 matmul (-2<a,b> + |a|^2 + |b|^2), sqrt and exp on the ACT engine.
Softmax skips the max-subtraction (scores are O(5), no overflow risk in
fp32); normalization is deferred past attn@V into the output projection
as a per-row scale. All operands are pre-transposed host-side so the PE
contracts over partition dims with no on-device input transposes; only
the attention matrix itself is transposed on the PE (via identity).
"""

import os
import sys
import numpy as np

if "/opt/trn_rl_repo" not in sys.path:
    sys.path.insert(0, "/opt/trn_rl_repo")

D = 256
B = 32
NCORES = 8
SLOTS = B // NCORES  # 4 batches per core
RL = 128             # padded lig rows per batch slot (1 partition chunk)
NEG = -1.0e5
EPS = 1e-5
SCALE = 1.0 / 16.0   # 1/sqrt(D)

_PROG_CACHE = {}


def _w_t(w):
    # W [D, D] -> lhsT layout [128, 2, D]: arr[p, h, d] = W[d, 128h+p]
    return np.ascontiguousarray(
        w.T.reshape(2, 128, D).transpose(1, 0, 2)).astype(np.float32)


def _t_pack(x_pad, width):
    # x_pad [S, width, D] -> [128, 2, S*width]: arr[p,h,s*width+c] = x[s,c,128h+p]
    s = x_pad.shape[0]
    t = x_pad.transpose(2, 0, 1).reshape(2, 128, s, width)
    return np.ascontiguousarray(t.transpose(1, 0, 2, 3).reshape(128, 2, s * width))


def _prepare(inputs):
    f32 = lambda k: np.ascontiguousarray(np.asarray(inputs[k], dtype=np.float32))
    lig, pro = f32("lig"), f32("pro")
    lpos, ppos = f32("lig_pos"), f32("pro_pos")
    lb = np.asarray(inputs["lig_batch"]).astype(np.int64).ravel()
    pb = np.asarray(inputs["pro_batch"]).astype(np.int64).ravel()
    NL, NP = lig.shape[0], pro.shape[0]
    lperm = np.argsort(lb, kind="stable")
    pperm = np.argsort(pb, kind="stable")
    lb_s, pb_s = lb[lperm], pb[pperm]
    lig_s, lpos_s = lig[lperm], lpos[lperm]
    pro_s, ppos_s = pro[pperm], ppos[pperm]
    nl = np.bincount(lb_s, minlength=B).astype(np.int64)
    npb = np.bincount(pb_s, minlength=B).astype(np.int64)
    assert nl.max() <= RL, f"batch lig rows {nl.max()} > {RL}"
    assert len(nl) == B and len(npb) == B
    CPC = max(128, int(-(-int(npb.max()) // 128)) * 128)
    NCH = CPC // 128
    lstart = np.zeros(B + 1, np.int64)
    lstart[1:] = np.cumsum(nl)
    pstart = np.zeros(B + 1, np.int64)
    pstart[1:] = np.cumsum(npb)

    bout_l = f32("bout_lig").reshape(1, D)
    bout_p = f32("bout_pro").reshape(1, D)
    g_l, b_l = f32("g_lig").ravel(), f32("b_lig").ravel()
    g_p, b_p = f32("g_pro").ravel(), f32("b_pro").ravel()
    triv_l = bool(np.all(g_l == 1.0) and np.all(b_l == 0.0))
    triv_p = bool(np.all(g_p == 1.0) and np.all(b_p == 0.0))

    shared = {
        "wql": _w_t(f32("Wq_lig")), "wkp": _w_t(f32("Wk_pro")),
        "wvp": _w_t(f32("Wv_pro")), "wqp": _w_t(f32("Wq_pro")),
        "wkl": _w_t(f32("Wk_lig")), "wvl": _w_t(f32("Wv_lig")),
        "wol": _w_t(f32("Wout_lig")), "wop": _w_t(f32("Wout_pro")),
        "gl": np.ascontiguousarray(np.broadcast_to(g_l, (128, D))),
        "bl": np.ascontiguousarray(np.broadcast_to(b_l, (128, D))),
        "gp": np.ascontiguousarray(np.broadcast_to(g_p, (128, D))),
        "bp": np.ascontiguousarray(np.broadcast_to(b_p, (128, D))),
    }

    in_maps = []
    for c in range(NCORES):
        bs = [SLOTS * c + s for s in range(SLOTS)]
        lig_pad = np.zeros((SLOTS, RL, D), np.float32)
        lpos_pad = np.zeros((SLOTS, RL, 3), np.float32)
        pro_pad = np.zeros((SLOTS, CPC, D), np.float32)
        ppos_pad = np.zeros((SLOTS, CPC, 3), np.float32)
        mpro = np.full((SLOTS, CPC), NEG, np.float32)
        mlig = np.full((SLOTS, RL), NEG, np.float32)
        for s, b in enumerate(bs):
            ln, pn = int(nl[b]), int(npb[b])
            lig_pad[s, :ln] = lig_s[lstart[b]:lstart[b + 1]]
            lpos_pad[s, :ln] = lpos_s[lstart[b]:lstart[b + 1]]
            pro_pad[s, :pn] = pro_s[pstart[b]:pstart[b + 1]]
            ppos_pad[s, :pn] = ppos_s[pstart[b]:pstart[b + 1]]
            mpro[s, :pn] = 0.0
            mlig[s, :ln] = 0.0
        na = (lpos_pad ** 2).sum(-1)          # [S, RL]
        nb = (ppos_pad ** 2).sum(-1)          # [S, CPC]
        one_l = np.ones_like(na)
        one_p = np.ones_like(nb)
        lx, ly, lz = lpos_pad[..., 0], lpos_pad[..., 1], lpos_pad[..., 2]
        px, py, pz = ppos_pad[..., 0], ppos_pad[..., 1], ppos_pad[..., 2]
        m = {
            # residual inputs carry bout folded in
            "ligx": np.ascontiguousarray(lig_pad.transpose(1, 0, 2) + bout_l[None]),
            "prox": np.ascontiguousarray(
                pro_pad.reshape(SLOTS, NCH, 128, D).transpose(2, 0, 1, 3)
                .reshape(128, SLOTS * NCH, D) + bout_p[None]),
            "ligT": _t_pack(lig_pad, RL),
            "proT": _t_pack(pro_pad, CPC),
            "lposA": np.ascontiguousarray(
                np.stack([-2 * lx, -2 * ly, -2 * lz, one_l, na])),   # [5,S,RL]
            "lposB": np.ascontiguousarray(
                np.stack([lx, ly, lz, na, one_l])),                  # [5,S,RL]
            "pposA": np.ascontiguousarray(
                np.stack([-2 * px, -2 * py, -2 * pz, one_p, nb])),   # [5,S,CPC]
            "pposB": np.ascontiguousarray(
                np.stack([px, py, pz, nb, one_p])),                  # [5,S,CPC]
            "mpro": np.ascontiguousarray(mpro[None]),                # [1,S,CPC]
            "mlig": np.ascontiguousarray(mlig[None]),                # [1,S,RL]
        }
        m.update(shared)
        in_maps.append(m)

    meta = dict(NL=NL, NP=NP, CPC=CPC, NCH=NCH, nl=nl, npb=npb,
                lstart=lstart, pstart=pstart, lperm=lperm, pperm=pperm,
                triv_l=triv_l, triv_p=triv_p)
    return in_maps, meta


def _unpack(results, meta):
    NL, NP = meta["NL"], meta["NP"]
    NCH = meta["NCH"]
    nl, npb = meta["nl"], meta["npb"]
    lstart, pstart = meta["lstart"], meta["pstart"]
    lperm, pperm = meta["lperm"], meta["pperm"]
    lig_out = np.zeros((NL, D), np.float32)
    pro_out = np.zeros((NP, D), np.float32)
    for c in range(NCORES):
        ligy = results[c]["ligy"]   # [SLOTS, 128, D]
        proy = results[c]["proy"]   # [SLOTS*NCH, 128, D]
        for s in range(SLOTS):
            b = SLOTS * c + s
            ln, pn = int(nl[b]), int(npb[b])
            if ln > 0:
                lig_out[lperm[lstart[b]:lstart[b + 1]]] = ligy[s, :ln, :]
            for j in range(NCH):
                r0 = j * 128
                n = min(128, pn - r0)
                if n > 0:
                    idx = pperm[pstart[b] + r0: pstart[b] + r0 + n]
                    pro_out[idx] = proy[s * NCH + j, :n, :]
    return lig_out, pro_out


def _numpy_core(m, CPC, NCH, triv_l, triv_p):
    """Numpy mirror of the device program (one core). For validation."""
    def ln(x, g, b):
        mu = x.mean(-1, keepdims=True)
        var = ((x - mu) ** 2).mean(-1, keepdims=True)
        return (x - mu) / np.sqrt(var + EPS) * g + b

    ligy = np.zeros((SLOTS, 128, D), np.float32)
    proy = np.zeros((SLOTS * NCH, 128, D), np.float32)
    # reassemble per-core padded operands
    ligT = m["ligT"].transpose(1, 0, 2).reshape(D, SLOTS, RL)      # [D,S,RL]
    proT = m["proT"].transpose(1, 0, 2).reshape(D, SLOTS, CPC)
    wt = {k: m[k].transpose(1, 0, 2).reshape(D, D) for k in
          ["wql", "wkp", "wvp", "wqp", "wkl", "wvl", "wol", "wop"]}
    gl, bl, gp, bp = m["gl"][0], m["bl"][0], m["gp"][0], m["bp"][0]
    for s in range(SLOTS):
        lig_s = ligT[:, s, :].T                                    # [RL, D]
        pro_s = proT[:, s, :].T                                    # [CPC, D]
        QT = wt["wql"].T @ ligT[:, s, :] * SCALE                   # [D, RL]
        K2T = wt["wkl"].T @ ligT[:, s, :]
        KT = wt["wkp"].T @ proT[:, s, :]                           # [D, CPC]
        Q2T = wt["wqp"].T @ proT[:, s, :] * SCALE
        V = pro_s @ wt["wvp"]                                      # [CPC, D]
        V2 = lig_s @ wt["wvl"]                                     # [RL, D]
        d2 = (m["lposA"][:, s, :].T @ m["pposB"][:, s, :])         # [RL, CPC]
        bias = np.exp(-np.sqrt(np.maximum(d2, 1e-12)) / 10.0)
        S1 = QT.T @ KT + np.ones((RL, 1), np.float32) @ m["mpro"][:, s, :]
        E = np.exp(S1 + bias)
        rec = 1.0 / E.sum(-1, keepdims=True)
        ctxT = V.T @ E.T                                           # [D, RL]
        z = (ctxT.T @ wt["wol"]) * rec + m["ligx"][:, s, :]
        ligy[s] = ln(z, gl, bl)
        # dir-2
        d2t = (m["pposA"][:, s, :].T @ m["lposB"][:, s, :])        # [CPC, RL]
        bias2 = np.exp(-np.sqrt(np.maximum(d2t, 1e-12)) / 10.0)
        S2 = Q2T.T @ K2T + np.ones((CPC, 1), np.float32) @ m["mlig"][:, s, :]
        E2 = np.exp(S2 + bias2)
        rec2 = 1.0 / E2.sum(-1, keepdims=True)
        ctx2T = V2.T @ E2.T                                        # [D, CPC]
        z2 = (ctx2T.T @ wt["wop"]) * rec2 + \
            m["prox"][:, s * NCH:(s + 1) * NCH, :].transpose(1, 0, 2).reshape(CPC, D)
        z2 = ln(z2, gp, bp)
        for j in range(NCH):
            proy[s * NCH + j] = z2[j * 128:(j + 1) * 128]
    return {"ligy": ligy, "proy": proy}


def _build_program(CPC, triv_l, triv_p):
    import concourse.mybir as mybir
    import concourse.tile as tile
    from concourse import bacc
    from concourse.masks import make_identity

    NCH = CPC // 128
    f32 = mybir.dt.float32
    AF = mybir.ActivationFunctionType
    OP = mybir.AluOpType

    nc = bacc.Bacc("TRN2", target_bir_lowering=False, debug=False,
                   num_devices=NCORES)

    din = {}
    for name, shape in [
        ("ligx", [128, SLOTS, D]), ("prox", [128, SLOTS * NCH, D]),
        ("ligT", [128, 2, SLOTS * RL]), ("proT", [128, 2, SLOTS * CPC]),
        ("lposA", [5, SLOTS, RL]), ("lposB", [5, SLOTS, RL]),
        ("pposA", [5, SLOTS, CPC]), ("pposB", [5, SLOTS, CPC]),
        ("mpro", [1, SLOTS, CPC]), ("mlig", [1, SLOTS, RL]),
        ("wql", [128, 2, D]), ("wkp", [128, 2, D]), ("wvp", [128, 2, D]),
        ("wqp", [128, 2, D]), ("wkl", [128, 2, D]), ("wvl", [128, 2, D]),
        ("wol", [128, 2, D]), ("wop", [128, 2, D]),
        ("gl", [128, D]), ("bl", [128, D]), ("gp", [128, D]), ("bp", [128, D]),
    ]:
        din[name] = nc.dram_tensor(name, shape, f32, kind="ExternalInput")
    ligy_d = nc.dram_tensor("ligy", [SLOTS, 128, D], f32, kind="ExternalOutput")
    proy_d = nc.dram_tensor("proy", [SLOTS * NCH, 128, D], f32,
                            kind="ExternalOutput")

    with tile.TileContext(nc) as tc:
        with tc.tile_pool(name="const", bufs=1) as cp, \
             tc.tile_pool(name="wk3", bufs=3) as wk3, \
             tc.tile_pool(name="wk2", bufs=2) as wk2, \
             tc.tile_pool(name="stat", bufs=16) as stp, \
             tc.tile_pool(name="psA", bufs=2, space="PSUM") as psA, \
             tc.tile_pool(name="psB", bufs=3, space="PSUM") as psB:

            def load(name):
                t = cp.tile(din[name].shape, f32, tag=name)
                nc.sync.dma_start(t[:], din[name].ap()[:])
                return t

            ligx, prox = load("ligx"), load("prox")
            ligT, proT = load("ligT"), load("proT")
            lposA, lposB = load("lposA"), load("lposB")
            pposA, pposB = load("pposA"), load("pposB")
            mpro, mlig = load("mpro"), load("mlig")
            wql, wkp, wvp = load("wql"), load("wkp"), load("wvp")
            wqp, wkl, wvl = load("wqp"), load("wkl"), load("wvl")
            wol, wop = load("wol"), load("wop")
            gl = load("gl") if not triv_l else None
            bl = load("bl") if not triv_l else None
            gp = load("gp") if not triv_p else None
            bp = load("bp") if not triv_p else None

            ident = cp.tile([128, 128], f32, tag="ident")
            make_identity(nc, ident[:])
            ones1 = cp.tile([1, 128], f32, tag="ones1")
            nc.vector.memset(ones1[:], 1.0)
            epsb = cp.tile([128, 1], f32, tag="epsb")
            nc.vector.memset(epsb[:], EPS)

            QT = cp.tile([128, 2, SLOTS * RL], f32, tag="QT")
            K2T = cp.tile([128, 2, SLOTS * RL], f32, tag="K2T")
            KT = cp.tile([128, 2, SLOTS * CPC], f32, tag="KT")
            Q2T = cp.tile([128, 2, SLOTS * CPC], f32, tag="Q2T")
            V = cp.tile([128, SLOTS * NCH, D], f32, tag="V")
            V2 = cp.tile([128, SLOTS, D], f32, tag="V2")

            def proj_t(dst, wt, src, width, scale):
                # dst[:, g, :] = sum_h wt[:,h,128g:+128].T @ src[:,h,:], scaled
                for g in range(2):
                    for n0 in range(0, width, 512):
                        n1 = min(n0 + 512, width)
                        ps = psA.tile([128, CPC], f32, tag="big")
                        for h in range(2):
                            nc.tensor.matmul(
                                ps[:, :n1 - n0], wt[:, h, 128 * g:128 * (g + 1)],
                                src[:, h, n0:n1], start=(h == 0), stop=(h == 1))
                        if scale != 1.0:
                            nc.scalar.activation(dst[:, g, n0:n1], ps[:, :n1 - n0],
                                                 AF.Copy, scale=scale)
                        else:
                            nc.vector.tensor_copy(dst[:, g, n0:n1], ps[:, :n1 - n0])

            def proj_n(dst, src, wt, nchunks):
                # dst[:, k, :] = src-rows(chunk k) @ W.T   (natural row layout)
                for k in range(nchunks):
                    ps = psB.tile([128, D], f32, tag="small")
                    for h in range(2):
                        nc.tensor.matmul(ps[:], src[:, h, 128 * k:128 * (k + 1)],
                                         wt[:, h, :], start=(h == 0), stop=(h == 1))
                    nc.vector.tensor_copy(dst[:, k, :], ps[:])

            proj_t(QT, wql, ligT, SLOTS * RL, SCALE)
            proj_t(K2T, wkl, ligT, SLOTS * RL, 1.0)
            proj_t(KT, wkp, proT, SLOTS * CPC, 1.0)
            proj_t(Q2T, wqp, proT, SLOTS * CPC, SCALE)
            proj_n(V, proT, wvp, SLOTS * NCH)
            proj_n(V2, ligT, wvl, SLOTS)

            def epilogue(zp, rec_ap, x_ap, g, b, out_ap):
                w = wk3.tile([128, D], f32, tag="w256")
                msum = stp.tile([128, 1], f32, tag="stat")
                nc.vector.scalar_tensor_tensor(
                    w[:], zp, rec_ap, x_ap, op0=OP.mult, op1=OP.add,
                    accum_out=msum[:])
                negmu = stp.tile([128, 1], f32, tag="stat")
                nc.scalar.activation(negmu[:], msum[:], AF.Copy, scale=-1.0 / D)
                wc = wk3.tile([128, D], f32, tag="w256")
                nc.scalar.activation(wc[:], w[:], AF.Identity, bias=negmu[:])
                sq = wk3.tile([128, D], f32, tag="w256")
                ssq = stp.tile([128, 1], f32, tag="stat")
                nc.scalar.activation(sq[:], wc[:], AF.Square, accum_out=ssq[:])
                stdt = stp.tile([128, 1], f32, tag="stat")
                nc.scalar.activation(stdt[:], ssq[:], AF.Sqrt, scale=1.0 / D,
                                     bias=epsb[:])
                rstd = stp.tile([128, 1], f32, tag="stat")
                nc.vector.reciprocal(rstd[:], stdt[:])
                o = wk3.tile([128, D], f32, tag="w256")
                if g is None:
                    nc.vector.tensor_scalar_mul(o[:], wc[:], rstd[:])
                else:
                    nc.vector.scalar_tensor_tensor(
                        o[:], wc[:], rstd[:], g[:], op0=OP.mult, op1=OP.mult)
                    nc.vector.tensor_tensor(o[:], o[:], b[:], OP.add)
                nc.sync.dma_start(out_ap, o[:])

            for s in range(SLOTS):
                # ---------------- dir-1: lig rows <- pro cols ----------------
                d2p = psA.tile([128, CPC], f32, tag="big")
                for n0 in range(0, CPC, 512):
                    n1 = min(n0 + 512, CPC)
                    nc.tensor.matmul(d2p[:, n0:n1], lposA[:, s, :],
                                     pposB[:, s, n0:n1], start=True, stop=True)
                d2c = wk3.tile([128, CPC], f32, tag="w640")
                nc.vector.tensor_scalar_max(d2c[:], d2p[:], 1e-12)
                dist = wk3.tile([128, CPC], f32, tag="w640")
                nc.scalar.activation(dist[:], d2c[:], AF.Sqrt)
                bias = wk3.tile([128, CPC], f32, tag="w640")
                nc.scalar.activation(bias[:], dist[:], AF.Exp, scale=-0.1)

                sp = psA.tile([128, CPC], f32, tag="big")
                for n0 in range(0, CPC, 512):
                    n1 = min(n0 + 512, CPC)
                    nc.tensor.matmul(sp[:, n0:n1], QT[:, 0, RL * s:RL * (s + 1)],
                                     KT[:, 0, CPC * s + n0:CPC * s + n1],
                                     start=True, stop=False)
                    nc.tensor.matmul(sp[:, n0:n1], QT[:, 1, RL * s:RL * (s + 1)],
                                     KT[:, 1, CPC * s + n0:CPC * s + n1],
                                     start=False, stop=False)
                    nc.tensor.matmul(sp[:, n0:n1], ones1[:],
                                     mpro[:, s, n0:n1], start=False, stop=True)
                ein = wk3.tile([128, CPC], f32, tag="w640")
                nc.vector.tensor_tensor(ein[:], sp[:], bias[:], OP.add)
                e1 = wk3.tile([128, CPC], f32, tag="w640")
                den = stp.tile([128, 1], f32, tag="stat")
                nc.scalar.activation(e1[:], ein[:], AF.Exp, accum_out=den[:])
                rec = stp.tile([128, 1], f32, tag="stat")
                nc.vector.reciprocal(rec[:], den[:])

                at = wk3.tile([128, NCH, 128], f32, tag="at")
                for j in range(NCH):
                    tp = psB.tile([128, 128], f32, tag="small")
                    nc.tensor.transpose(tp[:], e1[:, 128 * j:128 * (j + 1)],
                                        ident[:])
                    nc.vector.tensor_copy(at[:, j, :], tp[:])
                ctp = psB.tile([128, 2, 128], f32, tag="small")
                for h in range(2):
                    for j in range(NCH):
                        nc.tensor.matmul(
                            ctp[:, h, :], V[:, NCH * s + j, 128 * h:128 * (h + 1)],
                            at[:, j, :], start=(j == 0), stop=(j == NCH - 1))
                ctx = wk2.tile([128, 2, 128], f32, tag="ctx")
                nc.scalar.activation(ctx[:], ctp[:], AF.Copy)
                zp = psB.tile([128, D], f32, tag="small")
                for h in range(2):
                    nc.tensor.matmul(zp[:], ctx[:, h, :], wol[:, h, :],
                                     start=(h == 0), stop=(h == 1))
                epilogue(zp[:], rec[:], ligx[:, s, :], gl, bl,
                         ligy_d.ap()[s])

                # ---------------- dir-2: pro rows <- lig cols ----------------
                d2p2 = psA.tile([128, NCH, 128], f32, tag="big")
                for j in range(NCH):
                    nc.tensor.matmul(d2p2[:, j, :],
                                     pposA[:, s, 128 * j:128 * (j + 1)],
                                     lposB[:, s, :], start=True, stop=True)
                d2c2 = wk3.tile([128, NCH, 128], f32, tag="w640")
                nc.vector.tensor_scalar_max(d2c2[:], d2p2[:], 1e-12)
                dist2 = wk3.tile([128, NCH, 128], f32, tag="w640")
                nc.scalar.activation(dist2[:], d2c2[:], AF.Sqrt)
                bias2 = wk3.tile([128, NCH, 128], f32, tag="w640")
                nc.scalar.activation(bias2[:], dist2[:], AF.Exp, scale=-0.1)

                s2p = psA.tile([128, NCH, 128], f32, tag="big")
                for j in range(NCH):
                    c0 = CPC * s + 128 * j
                    nc.tensor.matmul(s2p[:, j, :], Q2T[:, 0, c0:c0 + 128],
                                     K2T[:, 0, RL * s:RL * s + 128],
                                     start=True, stop=False)
                    nc.tensor.matmul(s2p[:, j, :], Q2T[:, 1, c0:c0 + 128],
                                     K2T[:, 1, RL * s:RL * s + 128],
                                     start=False, stop=False)
                    nc.tensor.matmul(s2p[:, j, :], ones1[:], mlig[:, s, :],
                                     start=False, stop=True)
                ein2 = wk3.tile([128, NCH, 128], f32, tag="w640")
                nc.vector.tensor_tensor(ein2[:], s2p[:], bias2[:], OP.add)
                e2 = wk3.tile([128, NCH, 128], f32, tag="w640")
                den2 = stp.tile([128, NCH], f32, tag="statN")
                for j in range(NCH):
                    nc.scalar.activation(e2[:, j, :], ein2[:, j, :], AF.Exp,
                                         accum_out=den2[:, j:j + 1])
                rec2 = stp.tile([128, NCH], f32, tag="statN")
                nc.vector.reciprocal(rec2[:], den2[:])

                at2 = wk3.tile([128, NCH, 128], f32, tag="at")
                for j in range(NCH):
                    tp = psB.tile([128, 128], f32, tag="small")
                    nc.tensor.transpose(tp[:], e2[:, j, :], ident[:])
                    nc.vector.tensor_copy(at2[:, j, :], tp[:])
                ctx2 = wk2.tile([128, 2, CPC], f32, tag="c2s")
                for h in range(2):
                    c2p = psA.tile([128, CPC], f32, tag="big")
                    for j0 in range(0, NCH, 4):
                        j1 = min(j0 + 4, NCH)
                        nc.tensor.matmul(c2p[:, 128 * j0:128 * j1],
                                         V2[:, s, 128 * h:128 * (h + 1)],
                                         at2[:, j0:j1, :], start=True, stop=True)
                    nc.scalar.activation(ctx2[:, h, :], c2p[:], AF.Copy)
                for j in range(NCH):
                    zp2 = psB.tile([128, D], f32, tag="small")
                    for h in range(2):
                        nc.tensor.matmul(zp2[:], ctx2[:, h, 128 * j:128 * (j + 1)],
                                         wop[:, h, :], start=(h == 0), stop=(h == 1))
                    epilogue(zp2[:], rec2[:, j:j + 1], prox[:, NCH * s + j, :],
                             gp, bp, proy_d.ap()[NCH * s + j])

    nc.compile()
    return nc


def _ensure_ntff_hook():
    """Register the axon NTFF profiling hook if the image lacks
    antenv.axon_hooks (bass_utils imports it when trace=True)."""
    try:
        from antenv.axon_hooks import get_axon_ntff_profile_hook  # noqa: F401
        return
    except ImportError:
        pass
    import types
    import antenv
    mod = types.ModuleType("antenv.axon_hooks")
    state = {"h": None}
    mod.set_axon_ntff_profile_hook = lambda h: state.__setitem__("h", h)
    mod.get_axon_ntff_profile_hook = lambda: state["h"]
    sys.modules["antenv.axon_hooks"] = mod
    antenv.axon_hooks = mod
    try:
        from trn_agent_boot.trn_boot import _ntff_profile_via_ctypes
        mod.set_axon_ntff_profile_hook(
            _ntff_profile_via_ctypes("/opt/axon/libaxon_pjrt.so"))
    except Exception:
        pass


def _run_device(in_maps, meta, trace=False):
    if trace:
        _ensure_ntff_hook()
    from concourse.bass_utils import run_bass_kernel_spmd
    key = (meta["CPC"], meta["triv_l"], meta["triv_p"])
    if key not in _PROG_CACHE:
        _PROG_CACHE[key] = _build_program(*key)
    nc = _PROG_CACHE[key]
    res = run_bass_kernel_spmd(nc, in_maps, core_ids=list(range(NCORES)),
                               trace=trace)
    return res


def kernel(**inputs):
    in_maps, meta = _prepare(inputs)
    if os.environ.get("KERNEL_NUMPY"):
        results = [_numpy_core(m, meta["CPC"], meta["NCH"],
                               meta["triv_l"], meta["triv_p"])
                   for m in in_maps]
    else:
        results = _run_device(in_maps, meta).results
    return _unpack(results, meta)


def kernel_traced(**inputs):
    """Like kernel() but returns (outputs, BassKernelResults) with profiling."""
    in_maps, meta = _prepare(inputs)
    res = _run_device(in_maps, meta, trace=True)
    return _unpack(res.results, meta), res


# revision 8
# speedup vs baseline: 1.5277x; 1.5277x over previous
"""Trainium2 Bass kernel: bidirectional ligand<->protein cross-attention.

Batch ids are sorted, so the lig/pro attention mask is block-diagonal over
the 32 batches. Each core gets 4 batches, padded to uniform slots (lig
rows -> 128, pro rows -> CPC), one SPMD program for all 8 cores.

Key structure per slot (lig rows r=128 on partitions, pro cols c=CPC):
  d2   [r,c]  K=5 matmul (-2<a,b> + |a|^2 + |b|^2), sqrt+exp -> bias
              (shared by BOTH directions: bias2^T == bias)
  dir1: S = (Q/16)K^T + onesxmask (rank-1 mask matmul), E = exp(S+bias)
        with row-sum accum; A^T via PE transposes; ctx = A^T^T... row
        layout; z = ctx @ Wout^T via transpose of ctx; normalize by 1/den
        folded into the epilogue as a per-partition scale.
  dir2: S2T [lig, pro] = K2^T Q2/16 (transposed layout!) so the lig-pad
        mask is a per-partition ACT bias and the shared dist bias adds
        directly; E2T feeds ctx2T and a ones-matmul for denominators.
Matmuls with free dim >= 256 use float32r (1 cyc/row vs fp32's 4).
"""

import os
import sys
import numpy as np

if "/opt/trn_rl_repo" not in sys.path:
    sys.path.insert(0, "/opt/trn_rl_repo")

D = 256
B = 32
NCORES = 8
SLOTS = B // NCORES  # 4 batches per core
RL = 128             # padded lig rows per batch slot (1 partition chunk)
NEG = -1.0e5
EPS = 1e-5
SCALE = 1.0 / 16.0   # 1/sqrt(D)

_PROG_CACHE = {}


def _w_t(w):
    # W [D, D] -> lhsT layout [128, 2, D]: arr[p, h, d] = W[d, 128h+p]
    return np.ascontiguousarray(
        w.T.reshape(2, 128, D).transpose(1, 0, 2)).astype(np.float32)


def _t_pack(x_pad, width):
    # x_pad [S, width, D] -> [128, 2, S*width]: arr[p,h,s*width+c] = x[s,c,128h+p]
    s = x_pad.shape[0]
    t = x_pad.transpose(2, 0, 1).reshape(2, 128, s, width)
    return np.ascontiguousarray(t.transpose(1, 0, 2, 3).reshape(128, 2, s * width))


def _prepare(inputs):
    f32 = lambda k: np.ascontiguousarray(np.asarray(inputs[k], dtype=np.float32))
    lig, pro = f32("lig"), f32("pro")
    lpos, ppos = f32("lig_pos"), f32("pro_pos")
    lb = np.asarray(inputs["lig_batch"]).astype(np.int64).ravel()
    pb = np.asarray(inputs["pro_batch"]).astype(np.int64).ravel()
    NL, NP = lig.shape[0], pro.shape[0]
    lperm = np.argsort(lb, kind="stable")
    pperm = np.argsort(pb, kind="stable")
    lb_s, pb_s = lb[lperm], pb[pperm]
    lig_s, lpos_s = lig[lperm], lpos[lperm]
    pro_s, ppos_s = pro[pperm], ppos[pperm]
    nl = np.bincount(lb_s, minlength=B).astype(np.int64)
    npb = np.bincount(pb_s, minlength=B).astype(np.int64)
    assert nl.max() <= RL, f"batch lig rows {nl.max()} > {RL}"
    CPC = max(128, int(-(-int(npb.max()) // 128)) * 128)
    NCH = CPC // 128
    lstart = np.zeros(B + 1, np.int64)
    lstart[1:] = np.cumsum(nl)
    pstart = np.zeros(B + 1, np.int64)
    pstart[1:] = np.cumsum(npb)

    bout_l = f32("bout_lig").reshape(1, D)
    bout_p = f32("bout_pro").reshape(1, D)
    g_l, b_l = f32("g_lig").ravel(), f32("b_lig").ravel()
    g_p, b_p = f32("g_pro").ravel(), f32("b_pro").ravel()
    triv_l = bool(np.all(g_l == 1.0) and np.all(b_l == 0.0))
    triv_p = bool(np.all(g_p == 1.0) and np.all(b_p == 0.0))

    shared = {
        "wql": _w_t(f32("Wq_lig")), "wkp": _w_t(f32("Wk_pro")),
        "wvp": _w_t(f32("Wv_pro")), "wqp": _w_t(f32("Wq_pro")),
        "wkl": _w_t(f32("Wk_lig")), "wvl": _w_t(f32("Wv_lig")),
        "wol": _w_t(f32("Wout_lig")), "wop": _w_t(f32("Wout_pro")),
        "onesr": np.ones((1, 128), np.float32),
        "onesc": np.ones((128, 1), np.float32),
        "gl": np.ascontiguousarray(np.broadcast_to(g_l, (128, D))),
        "bl": np.ascontiguousarray(np.broadcast_to(b_l, (128, D))),
        "gp": np.ascontiguousarray(np.broadcast_to(g_p, (128, D))),
        "bp": np.ascontiguousarray(np.broadcast_to(b_p, (128, D))),
    }

    in_maps = []
    for c in range(NCORES):
        bs = [SLOTS * c + s for s in range(SLOTS)]
        lig_pad = np.zeros((SLOTS, RL, D), np.float32)
        lpos_pad = np.zeros((SLOTS, RL, 3), np.float32)
        pro_pad = np.zeros((SLOTS, CPC, D), np.float32)
        ppos_pad = np.zeros((SLOTS, CPC, 3), np.float32)
        mpro = np.full((SLOTS, CPC), NEG, np.float32)
        mligP = np.full((128, SLOTS), NEG, np.float32)
        for s, b in enumerate(bs):
            ln, pn = int(nl[b]), int(npb[b])
            lig_pad[s, :ln] = lig_s[lstart[b]:lstart[b + 1]]
            lpos_pad[s, :ln] = lpos_s[lstart[b]:lstart[b + 1]]
            pro_pad[s, :pn] = pro_s[pstart[b]:pstart[b + 1]]
            ppos_pad[s, :pn] = ppos_s[pstart[b]:pstart[b + 1]]
            mpro[s, :pn] = 0.0
            mligP[:ln, s] = 0.0
        na = (lpos_pad ** 2).sum(-1)          # [S, RL]
        nb = (ppos_pad ** 2).sum(-1)          # [S, CPC]
        lx, ly, lz = lpos_pad[..., 0], lpos_pad[..., 1], lpos_pad[..., 2]
        px, py, pz = ppos_pad[..., 0], ppos_pad[..., 1], ppos_pad[..., 2]
        m = {
            # residual inputs carry bout folded in
            "ligx": np.ascontiguousarray(lig_pad.transpose(1, 0, 2) + bout_l[None]),
            "prox": np.ascontiguousarray(
                pro_pad.reshape(SLOTS, NCH, 128, D).transpose(2, 0, 1, 3)
                .reshape(128, SLOTS * NCH, D) + bout_p[None]),
            "ligT": _t_pack(lig_pad, RL),
            "proT": _t_pack(pro_pad, CPC),
            "lposA": np.ascontiguousarray(
                np.stack([-2 * lx, -2 * ly, -2 * lz, np.ones_like(na), na])),
            "pposB": np.ascontiguousarray(
                np.stack([px, py, pz, nb, np.ones_like(nb)])),
            "mpro": np.ascontiguousarray(mpro[None]),                # [1,S,CPC]
            "mligP": np.ascontiguousarray(mligP),                    # [128,S]
        }
        m.update(shared)
        in_maps.append(m)

    meta = dict(NL=NL, NP=NP, CPC=CPC, NCH=NCH, nl=nl, npb=npb,
                lstart=lstart, pstart=pstart, lperm=lperm, pperm=pperm,
                triv_l=triv_l, triv_p=triv_p)
    return in_maps, meta


def _unpack(results, meta):
    NL, NP = meta["NL"], meta["NP"]
    NCH = meta["NCH"]
    nl, npb = meta["nl"], meta["npb"]
    lstart, pstart = meta["lstart"], meta["pstart"]
    lperm, pperm = meta["lperm"], meta["pperm"]
    lig_out = np.zeros((NL, D), np.float32)
    pro_out = np.zeros((NP, D), np.float32)
    for c in range(NCORES):
        ligy = results[c]["ligy"]   # [SLOTS, 128, D]
        proy = results[c]["proy"]   # [SLOTS*NCH, 128, D]
        for s in range(SLOTS):
            b = SLOTS * c + s
            ln, pn = int(nl[b]), int(npb[b])
            if ln > 0:
                lig_out[lperm[lstart[b]:lstart[b + 1]]] = ligy[s, :ln, :]
            for j in range(NCH):
                r0 = j * 128
                n = min(128, pn - r0)
                if n > 0:
                    idx = pperm[pstart[b] + r0: pstart[b] + r0 + n]
                    pro_out[idx] = proy[s * NCH + j, :n, :]
    return lig_out, pro_out


def _numpy_core(m, CPC, NCH, triv_l, triv_p):
    """Numpy mirror of the device program (one core). For validation."""
    def ln(x, g, b):
        mu = x.mean(-1, keepdims=True)
        var = ((x - mu) ** 2).mean(-1, keepdims=True)
        return (x - mu) / np.sqrt(var + EPS) * g + b

    ligy = np.zeros((SLOTS, 128, D), np.float32)
    proy = np.zeros((SLOTS * NCH, 128, D), np.float32)
    ligT = m["ligT"].transpose(1, 0, 2).reshape(D, SLOTS, RL)      # [D,S,RL]
    proT = m["proT"].transpose(1, 0, 2).reshape(D, SLOTS, CPC)
    wt = {k: m[k].transpose(1, 0, 2).reshape(D, D) for k in
          ["wql", "wkp", "wvp", "wqp", "wkl", "wvl", "wol", "wop"]}
    gl, bl, gp, bp = m["gl"][0], m["bl"][0], m["gp"][0], m["bp"][0]
    for s in range(SLOTS):
        lig_s = ligT[:, s, :].T                                    # [RL, D]
        pro_s = proT[:, s, :].T                                    # [CPC, D]
        QT = wt["wql"].T @ ligT[:, s, :] * SCALE                   # [D, RL]
        K2T = wt["wkl"].T @ ligT[:, s, :]
        KT = wt["wkp"].T @ proT[:, s, :]                           # [D, CPC]
        Q2T = wt["wqp"].T @ proT[:, s, :] * SCALE
        V = pro_s @ wt["wvp"]                                      # [CPC, D]
        V2 = lig_s @ wt["wvl"]                                     # [RL, D]
        d2 = (m["lposA"][:, s, :].T @ m["pposB"][:, s, :])         # [RL, CPC]
        bias = np.exp(-np.sqrt(np.maximum(d2, 1e-12)) / 10.0)
        # dir1
        S1 = QT.T @ KT + np.ones((RL, 1), np.float32) @ m["mpro"][:, s, :]
        E = np.exp(S1 + bias)
        rec = 1.0 / E.sum(-1, keepdims=True)
        ctx = E @ V                                                # [RL, D]
        z = (ctx @ wt["wol"]) * rec + m["ligx"][:, s, :]
        ligy[s] = ln(z, gl, bl)
        # dir2 (transposed score layout)
        S2T = K2T.T @ Q2T + m["mligP"][:, s:s + 1]                 # [RL, CPC]
        E2T = np.exp(S2T + bias)
        den2 = E2T.sum(axis=0)                                     # [CPC]
        ctx2T = V2.T @ E2T                                         # [D, CPC]
        z2 = (ctx2T.T @ wt["wop"]) / den2[:, None] + \
            m["prox"][:, s * NCH:(s + 1) * NCH, :].transpose(1, 0, 2).reshape(CPC, D)
        z2 = ln(z2, gp, bp)
        for j in range(NCH):
            proy[s * NCH + j] = z2[j * 128:(j + 1) * 128]
    return {"ligy": ligy, "proy": proy}


def _build_program(CPC, triv_l, triv_p):
    import concourse.mybir as mybir
    import concourse.tile as tile
    from concourse import bacc
    from concourse.masks import make_identity

    NCH = CPC // 128
    f32 = mybir.dt.float32
    f32r = mybir.dt.float32r
    AF = mybir.ActivationFunctionType
    OP = mybir.AluOpType

    nc = bacc.Bacc("TRN2", target_bir_lowering=False, debug=False,
                   num_devices=NCORES)

    # f32r DRAM tensors carry plain fp32 bits; declaring them f32r lets
    # plain DMAs land in f32r SBUF tiles with no cast.
    din = {}
    for name, shape, dt_ in [
        ("ligx", [128, SLOTS, D], f32), ("prox", [128, SLOTS * NCH, D], f32),
        ("ligT", [128, 2, SLOTS * RL], f32r), ("proT", [128, 2, SLOTS * CPC], f32r),
        ("lposA", [5, SLOTS, RL], f32r), ("pposB", [5, SLOTS, CPC], f32r),
        ("mpro", [1, SLOTS, CPC], f32r), ("mligP", [128, SLOTS], f32),
        ("onesr", [1, 128], f32r), ("onesc", [128, 1], f32r),
        ("wql", [128, 2, D], f32r), ("wkp", [128, 2, D], f32r),
        ("wvp", [128, 2, D], f32r), ("wqp", [128, 2, D], f32r),
        ("wkl", [128, 2, D], f32r), ("wvl", [128, 2, D], f32r),
        ("wol", [128, 2, D], f32r), ("wop", [128, 2, D], f32r),
        ("gl", [128, D], f32), ("bl", [128, D], f32),
        ("gp", [128, D], f32), ("bp", [128, D], f32),
    ]:
        din[name] = nc.dram_tensor(name, shape, dt_, kind="ExternalInput")
    ligy_d = nc.dram_tensor("ligy", [SLOTS, 128, D], f32, kind="ExternalOutput")
    proy_d = nc.dram_tensor("proy", [SLOTS * NCH, 128, D], f32,
                            kind="ExternalOutput")

    # 512/128 col splits (psum-bank aligned)
    splits = [(n0, min(n0 + 512, CPC)) for n0 in range(0, CPC, 512)]

    with tile.TileContext(nc) as tc:
        with tc.tile_pool(name="const", bufs=1) as cp, \
             tc.tile_pool(name="slotin", bufs=2) as sip, \
             tc.tile_pool(name="slotp", bufs=2) as spp, \
             tc.tile_pool(name="wk3", bufs=3) as wk3, \
             tc.tile_pool(name="wk2", bufs=2) as wk2, \
             tc.tile_pool(name="stat", bufs=16) as stp, \
             tc.tile_pool(name="psA", bufs=2, space="PSUM") as psA, \
             tc.tile_pool(name="psB", bufs=3, space="PSUM") as psB, \
             tc.tile_pool(name="psD", bufs=1, space="PSUM") as psD:

            def load(name, dt_=None, eng=None):
                t = cp.tile(din[name].shape, dt_ or f32, tag=name)
                (eng or nc.sync).dma_start(t[:], din[name].ap()[:])
                return t

            # upfront loads (lig side + weights + consts)
            ligx = load("ligx")
            ligT = load("ligT", f32r, nc.gpsimd)
            lposA = load("lposA", f32r)
            mligP = load("mligP")
            onesr = load("onesr", f32r)
            onesc = load("onesc", f32r)
            wql = load("wql", f32r, nc.scalar)
            wkp = load("wkp", f32r, nc.scalar)
            wvp = load("wvp", f32r, nc.scalar)
            wqp = load("wqp", f32r, nc.scalar)
            wkl = load("wkl", f32r, nc.scalar)
            wvl = load("wvl", f32r, nc.scalar)
            wol = load("wol", f32r, nc.scalar)
            wop = load("wop", f32r, nc.scalar)
            gl = load("gl") if not triv_l else None
            bl = load("bl") if not triv_l else None
            gp = load("gp") if not triv_p else None
            bp = load("bp") if not triv_p else None

            ident = cp.tile([128, 128], f32, tag="ident")
            make_identity(nc, ident[:])
            epsb = cp.tile([128, 1], f32, tag="epsb")
            nc.vector.memset(epsb[:], EPS)

            # lig-side projections (small, upfront); all-slot width
            QT = cp.tile([128, 2, SLOTS * RL], f32r, tag="QT")
            K2T = cp.tile([128, 2, SLOTS * RL], f32r, tag="K2T")
            V2 = cp.tile([128, SLOTS, D], f32r, tag="V2")
            for g in range(2):
                ps = psA.tile([128, CPC], f32, tag="big")
                for h in range(2):
                    nc.tensor.matmul(ps[:, :SLOTS * RL], wql[:, h, 128 * g:128 * (g + 1)],
                                     ligT[:, h, :], start=(h == 0), stop=(h == 1))
                nc.scalar.activation(QT[:, g, :], ps[:, :SLOTS * RL], AF.Copy,
                                     scale=SCALE)
                ps2 = psA.tile([128, CPC], f32, tag="big")
                for h in range(2):
                    nc.tensor.matmul(ps2[:, :SLOTS * RL], wkl[:, h, 128 * g:128 * (g + 1)],
                                     ligT[:, h, :], start=(h == 0), stop=(h == 1))
                nc.vector.tensor_copy(K2T[:, g, :], ps2[:, :SLOTS * RL])
            for k in range(SLOTS):
                ps = psB.tile([128, D], f32, tag="small")
                for h in range(2):
                    nc.tensor.matmul(ps[:], ligT[:, h, 128 * k:128 * (k + 1)],
                                     wvl[:, h, :], start=(h == 0), stop=(h == 1))
                nc.vector.tensor_copy(V2[:, k, :], ps[:])

            def epilogue(zp, rec_ap, x_ap, g, b, out_ap):
                w = wk3.tile([128, D], f32, tag="w256")
                msum = stp.tile([128, 1], f32, tag="stat")
                nc.vector.scalar_tensor_tensor(
                    w[:], zp, rec_ap, x_ap, op0=OP.mult, op1=OP.add,
                    accum_out=msum[:])
                negmu = stp.tile([128, 1], f32, tag="stat")
                nc.vector.tensor_scalar_mul(negmu[:], msum[:], -1.0 / D)
                wc = wk3.tile([128, D], f32, tag="w256")
                nc.vector.tensor_scalar_add(wc[:], w[:], negmu[:])
                sq = wk3.tile([128, D], f32, tag="w256")
                ssq = stp.tile([128, 1], f32, tag="stat")
                nc.vector.scalar_tensor_tensor(
                    sq[:], wc[:], 1.0, wc[:], op0=OP.mult, op1=OP.mult,
                    accum_out=ssq[:])
                stdt = stp.tile([128, 1], f32, tag="stat")
                nc.scalar.activation(stdt[:], ssq[:], AF.Sqrt, scale=1.0 / D,
                                     bias=epsb[:])
                rstd = stp.tile([128, 1], f32, tag="stat")
                nc.vector.reciprocal(rstd[:], stdt[:])
                o = wk3.tile([128, D], f32, tag="w256")
                if g is None:
                    nc.vector.tensor_scalar_mul(o[:], wc[:], rstd[:])
                else:
                    nc.vector.scalar_tensor_tensor(
                        o[:], wc[:], rstd[:], g[:], op0=OP.mult, op1=OP.mult)
                    nc.vector.tensor_tensor(o[:], o[:], b[:], OP.add)
                nc.sync.dma_start(out_ap, o[:])

            dma_engs = [nc.sync, nc.scalar, nc.gpsimd, nc.sync]
            for s in range(SLOTS):
                # ---- per-slot pro-side loads ----
                proT_s = sip.tile([128, 2, CPC], f32r, tag="proT")
                dma_engs[s % 4].dma_start(
                    proT_s[:], din["proT"].ap()[:, :, CPC * s:CPC * (s + 1)])
                prox_s = sip.tile([128, NCH, D], f32, tag="prox")
                dma_engs[(s + 1) % 4].dma_start(
                    prox_s[:], din["prox"].ap()[:, NCH * s:NCH * (s + 1), :])
                pposB_s = sip.tile([5, CPC], f32r, tag="pposB")
                nc.sync.dma_start(pposB_s[:], din["pposB"].ap()[:, s, :])
                mpro_s = sip.tile([1, CPC], f32r, tag="mpro")
                nc.sync.dma_start(mpro_s[:], din["mpro"].ap()[:, s, :])

                # ---- per-slot pro-side projections ----
                KT_s = spp.tile([128, 2, CPC], f32r, tag="KT")
                Q2T_s = spp.tile([128, 2, CPC], f32r, tag="Q2T")
                for g in range(2):
                    ps = psA.tile([128, CPC], f32, tag="big")
                    for n0, n1 in splits:
                        for h in range(2):
                            nc.tensor.matmul(
                                ps[:, n0:n1], wkp[:, h, 128 * g:128 * (g + 1)],
                                proT_s[:, h, n0:n1], start=(h == 0), stop=(h == 1))
                    nc.vector.tensor_copy(KT_s[:, g, :], ps[:])
                    ps2 = psA.tile([128, CPC], f32, tag="big")
                    for n0, n1 in splits:
                        for h in range(2):
                            nc.tensor.matmul(
                                ps2[:, n0:n1], wqp[:, h, 128 * g:128 * (g + 1)],
                                proT_s[:, h, n0:n1], start=(h == 0), stop=(h == 1))
                    nc.scalar.activation(Q2T_s[:, g, :], ps2[:], AF.Copy,
                                         scale=SCALE)
                V_s = spp.tile([128, NCH, D], f32r, tag="V")
                for k in range(NCH):
                    ps = psB.tile([128, D], f32, tag="small")
                    for h in range(2):
                        nc.tensor.matmul(ps[:], proT_s[:, h, 128 * k:128 * (k + 1)],
                                         wvp[:, h, :], start=(h == 0), stop=(h == 1))
                    nc.vector.tensor_copy(V_s[:, k, :], ps[:])

                # ---- shared distance bias ----
                d2p = psA.tile([128, CPC], f32, tag="big")
                for n0, n1 in splits:
                    nc.tensor.matmul(d2p[:, n0:n1], lposA[:, s, :],
                                     pposB_s[:, n0:n1], start=True, stop=True)
                d2c = wk3.tile([128, CPC], f32, tag="w640")
                nc.vector.tensor_scalar_max(d2c[:], d2p[:], 1e-12)
                dist = wk3.tile([128, CPC], f32, tag="w640")
                nc.scalar.activation(dist[:], d2c[:], AF.Sqrt)
                bias = wk2.tile([128, CPC], f32, tag="bias")
                nc.scalar.activation(bias[:], dist[:], AF.Exp, scale=-0.1)

                # ---------------- dir-1: lig rows <- pro cols ----------------
                sp = psA.tile([128, CPC], f32, tag="big")
                for n0, n1 in splits:
                    nc.tensor.matmul(sp[:, n0:n1], QT[:, 0, RL * s:RL * (s + 1)],
                                     KT_s[:, 0, n0:n1], start=True, stop=False)
                    nc.tensor.matmul(sp[:, n0:n1], QT[:, 1, RL * s:RL * (s + 1)],
                                     KT_s[:, 1, n0:n1], start=False, stop=False)
                    nc.tensor.matmul(sp[:, n0:n1], onesr[:],
                                     mpro_s[:, n0:n1], start=False, stop=True)
                ein = wk3.tile([128, CPC], f32, tag="w640")
                nc.vector.tensor_tensor(ein[:], sp[:], bias[:], OP.add)
                e1 = wk3.tile([128, CPC], f32, tag="w640")
                den = stp.tile([128, 1], f32, tag="stat")
                nc.scalar.activation(e1[:], ein[:], AF.Exp, accum_out=den[:])
                rec = stp.tile([128, 1], f32, tag="stat")
                nc.vector.reciprocal(rec[:], den[:])

                at = wk3.tile([128, NCH, 128], f32r, tag="at")
                for j in range(NCH):
                    tp = psB.tile([128, 128], f32, tag="small")
                    nc.tensor.transpose(tp[:], e1[:, 128 * j:128 * (j + 1)],
                                        ident[:])
                    nc.vector.tensor_copy(at[:, j, :], tp[:])
                ctxp = psB.tile([128, D], f32, tag="small")
                for j in range(NCH):
                    nc.tensor.matmul(ctxp[:], at[:, j, :], V_s[:, j, :],
                                     start=(j == 0), stop=(j == NCH - 1))
                ctxs = wk2.tile([128, D], f32, tag="ctx")
                nc.scalar.activation(ctxs[:], ctxp[:], AF.Copy)
                ctxT = wk2.tile([128, 2, 128], f32r, tag="ctxT")
                for h in range(2):
                    tp = psB.tile([128, 128], f32, tag="small")
                    nc.tensor.transpose(tp[:], ctxs[:, 128 * h:128 * (h + 1)],
                                        ident[:])
                    nc.vector.tensor_copy(ctxT[:, h, :], tp[:])
                zp = psB.tile([128, D], f32, tag="small")
                for h in range(2):
                    nc.tensor.matmul(zp[:], ctxT[:, h, :], wol[:, h, :],
                                     start=(h == 0), stop=(h == 1))
                epilogue(zp[:], rec[:], ligx[:, s, :], gl, bl, ligy_d.ap()[s])

                # ------- dir-2 (transposed scores): pro rows <- lig cols -------
                s2p = psA.tile([128, CPC], f32, tag="big")
                for n0, n1 in splits:
                    nc.tensor.matmul(s2p[:, n0:n1], K2T[:, 0, RL * s:RL * (s + 1)],
                                     Q2T_s[:, 0, n0:n1], start=True, stop=False)
                    nc.tensor.matmul(s2p[:, n0:n1], K2T[:, 1, RL * s:RL * (s + 1)],
                                     Q2T_s[:, 1, n0:n1], start=False, stop=True)
                ein2 = wk3.tile([128, CPC], f32, tag="w640")
                nc.vector.tensor_tensor(ein2[:], s2p[:], bias[:], OP.add)
                e2t = wk3.tile([128, CPC], f32r, tag="e2t")
                nc.scalar.activation(e2t[:], ein2[:], AF.Exp,
                                     bias=mligP[:, s:s + 1])
                # denominators: column sums via ones matmul, then transpose
                dens = wk2.tile([1, CPC], f32, tag="dens")
                for n0, n1 in splits:
                    dp = psD.tile([1, 512], f32, tag="den")
                    nc.tensor.matmul(dp[:, :n1 - n0], onesc[:], e2t[:, n0:n1],
                                     start=True, stop=True)
                    nc.vector.tensor_copy(dens[:, n0:n1], dp[:, :n1 - n0])
                den2 = stp.tile([128, NCH], f32, tag="statN")
                for j in range(NCH):
                    dtp = psD.tile([128, 1], f32, tag="den")
                    nc.tensor.matmul(dtp[:], dens[:, 128 * j:128 * (j + 1)],
                                     ident[0:1, 0:1], start=True, stop=True)
                    nc.vector.tensor_copy(den2[:, j:j + 1], dtp[:])
                rec2 = stp.tile([128, NCH], f32, tag="statN")
                nc.vector.reciprocal(rec2[:], den2[:])

                ctx2T = wk2.tile([128, 2, CPC], f32r, tag="c2s")
                for h in range(2):
                    cp2 = psA.tile([128, CPC], f32, tag="big")
                    for n0, n1 in splits:
                        nc.tensor.matmul(cp2[:, n0:n1], V2[:, s, 128 * h:128 * (h + 1)],
                                         e2t[:, n0:n1], start=True, stop=True)
                    nc.scalar.activation(ctx2T[:, h, :], cp2[:], AF.Copy)
                for j in range(NCH):
                    zp2 = psB.tile([128, D], f32, tag="small")
                    for h in range(2):
                        nc.tensor.matmul(zp2[:], ctx2T[:, h, 128 * j:128 * (j + 1)],
                                         wop[:, h, :], start=(h == 0), stop=(h == 1))
                    epilogue(zp2[:], rec2[:, j:j + 1], prox_s[:, j, :],
                             gp, bp, proy_d.ap()[NCH * s + j])

    nc.compile()
    return nc


def _ensure_ntff_hook():
    """Register the axon NTFF profiling hook if the image lacks
    antenv.axon_hooks (bass_utils imports it when trace=True)."""
    try:
        from antenv.axon_hooks import get_axon_ntff_profile_hook  # noqa: F401
        return
    except ImportError:
        pass
    import types
    import antenv
    mod = types.ModuleType("antenv.axon_hooks")
    state = {"h": None}
    mod.set_axon_ntff_profile_hook = lambda h: state.__setitem__("h", h)
    mod.get_axon_ntff_profile_hook = lambda: state["h"]
    sys.modules["antenv.axon_hooks"] = mod
    antenv.axon_hooks = mod
    try:
        from trn_agent_boot.trn_boot import _ntff_profile_via_ctypes
        mod.set_axon_ntff_profile_hook(
            _ntff_profile_via_ctypes("/opt/axon/libaxon_pjrt.so"))
    except Exception:
        pass


def _run_device(in_maps, meta, trace=False):
    if trace:
        _ensure_ntff_hook()
    from concourse.bass_utils import run_bass_kernel_spmd
    key = (meta["CPC"], meta["triv_l"], meta["triv_p"])
    if key not in _PROG_CACHE:
        _PROG_CACHE[key] = _build_program(*key)
    nc = _PROG_CACHE[key]
    res = run_bass_kernel_spmd(nc, in_maps, core_ids=list(range(NCORES)),
                               trace=trace)
    return res


def kernel(**inputs):
    in_maps, meta = _prepare(inputs)
    if os.environ.get("KERNEL_NUMPY"):
        results = [_numpy_core(m, meta["CPC"], meta["NCH"],
                               meta["triv_l"], meta["triv_p"])
                   for m in in_maps]
    else:
        results = _run_device(in_maps, meta).results
    return _unpack(results, meta)


def kernel_traced(**inputs):
    """Like kernel() but returns (outputs, BassKernelResults) with profiling."""
    in_maps, meta = _prepare(inputs)
    res = _run_device(in_maps, meta, trace=True)
    return _unpack(res.results, meta), res


# revision 9
# speedup vs baseline: 1.5616x; 1.0222x over previous
"""Trainium2 Bass kernel: bidirectional ligand<->protein cross-attention.

Batch ids are sorted, so the lig/pro attention mask is block-diagonal over
the 32 batches. Each core gets 4 batches, padded to uniform slots (lig
rows -> 128, pro rows -> CPC), one SPMD program for all 8 cores.

Key structure per slot (lig rows r=128 on partitions, pro cols c=CPC):
  d2   [r,c]  K=5 matmul (-2<a,b> + |a|^2 + |b|^2), sqrt+exp -> bias
              (shared by BOTH directions: bias2^T == bias)
  dir1: S = (Q/16)K^T + onesxmask (rank-1 mask matmul), E = exp(S+bias)
        with row-sum accum; A^T via PE transposes; ctx = A^T^T... row
        layout; z = ctx @ Wout^T via transpose of ctx; normalize by 1/den
        folded into the epilogue as a per-partition scale.
  dir2: S2T [lig, pro] = K2^T Q2/16 (transposed layout!) so the lig-pad
        mask is a per-partition ACT bias and the shared dist bias adds
        directly; E2T feeds ctx2T and a ones-matmul for denominators.
Matmuls with free dim >= 256 use float32r (1 cyc/row vs fp32's 4).
"""

import os
import sys
import numpy as np

if "/opt/trn_rl_repo" not in sys.path:
    sys.path.insert(0, "/opt/trn_rl_repo")

D = 256
B = 32
NCORES = 8
SLOTS = B // NCORES  # 4 batches per core
RL = 128             # padded lig rows per batch slot (1 partition chunk)
NEG = -1.0e5
EPS = 1e-5
SCALE = 1.0 / 16.0   # 1/sqrt(D)

_PROG_CACHE = {}


def _w_t(w):
    # W [D, D] -> lhsT layout [128, 2, D]: arr[p, h, d] = W[d, 128h+p]
    return np.ascontiguousarray(
        w.T.reshape(2, 128, D).transpose(1, 0, 2)).astype(np.float32)


def _t_pack(x_pad, width):
    # x_pad [S, width, D] -> [128, 2, S*width]: arr[p,h,s*width+c] = x[s,c,128h+p]
    s = x_pad.shape[0]
    t = x_pad.transpose(2, 0, 1).reshape(2, 128, s, width)
    return np.ascontiguousarray(t.transpose(1, 0, 2, 3).reshape(128, 2, s * width))


def _prepare(inputs):
    f32 = lambda k: np.ascontiguousarray(np.asarray(inputs[k], dtype=np.float32))
    lig, pro = f32("lig"), f32("pro")
    lpos, ppos = f32("lig_pos"), f32("pro_pos")
    lb = np.asarray(inputs["lig_batch"]).astype(np.int64).ravel()
    pb = np.asarray(inputs["pro_batch"]).astype(np.int64).ravel()
    NL, NP = lig.shape[0], pro.shape[0]
    lperm = np.argsort(lb, kind="stable")
    pperm = np.argsort(pb, kind="stable")
    lb_s, pb_s = lb[lperm], pb[pperm]
    lig_s, lpos_s = lig[lperm], lpos[lperm]
    pro_s, ppos_s = pro[pperm], ppos[pperm]
    nl = np.bincount(lb_s, minlength=B).astype(np.int64)
    npb = np.bincount(pb_s, minlength=B).astype(np.int64)
    assert nl.max() <= RL, f"batch lig rows {nl.max()} > {RL}"
    CPC = max(128, int(-(-int(npb.max()) // 128)) * 128)
    NCH = CPC // 128
    lstart = np.zeros(B + 1, np.int64)
    lstart[1:] = np.cumsum(nl)
    pstart = np.zeros(B + 1, np.int64)
    pstart[1:] = np.cumsum(npb)

    bout_l = f32("bout_lig").reshape(1, D)
    bout_p = f32("bout_pro").reshape(1, D)
    g_l, b_l = f32("g_lig").ravel(), f32("b_lig").ravel()
    g_p, b_p = f32("g_pro").ravel(), f32("b_pro").ravel()
    triv_l = bool(np.all(g_l == 1.0) and np.all(b_l == 0.0))
    triv_p = bool(np.all(g_p == 1.0) and np.all(b_p == 0.0))

    shared = {
        "wql": _w_t(f32("Wq_lig")), "wkp": _w_t(f32("Wk_pro")),
        "wvp": _w_t(f32("Wv_pro")), "wqp": _w_t(f32("Wq_pro")),
        "wkl": _w_t(f32("Wk_lig")), "wvl": _w_t(f32("Wv_lig")),
        "wol": _w_t(f32("Wout_lig")), "wop": _w_t(f32("Wout_pro")),
        "onesr": np.ones((1, 128), np.float32),
        "onesc": np.ones((128, 1), np.float32),
        "gl": np.ascontiguousarray(np.broadcast_to(g_l, (128, D))),
        "bl": np.ascontiguousarray(np.broadcast_to(b_l, (128, D))),
        "gp": np.ascontiguousarray(np.broadcast_to(g_p, (128, D))),
        "bp": np.ascontiguousarray(np.broadcast_to(b_p, (128, D))),
    }

    in_maps = []
    for c in range(NCORES):
        bs = [SLOTS * c + s for s in range(SLOTS)]
        lig_pad = np.zeros((SLOTS, RL, D), np.float32)
        lpos_pad = np.zeros((SLOTS, RL, 3), np.float32)
        pro_pad = np.zeros((SLOTS, CPC, D), np.float32)
        ppos_pad = np.zeros((SLOTS, CPC, 3), np.float32)
        mpro = np.full((SLOTS, CPC), NEG, np.float32)
        mligP = np.full((128, SLOTS), NEG, np.float32)
        for s, b in enumerate(bs):
            ln, pn = int(nl[b]), int(npb[b])
            lig_pad[s, :ln] = lig_s[lstart[b]:lstart[b + 1]]
            lpos_pad[s, :ln] = lpos_s[lstart[b]:lstart[b + 1]]
            pro_pad[s, :pn] = pro_s[pstart[b]:pstart[b + 1]]
            ppos_pad[s, :pn] = ppos_s[pstart[b]:pstart[b + 1]]
            mpro[s, :pn] = 0.0
            mligP[:ln, s] = 0.0
        na = (lpos_pad ** 2).sum(-1)          # [S, RL]
        nb = (ppos_pad ** 2).sum(-1)          # [S, CPC]
        lx, ly, lz = lpos_pad[..., 0], lpos_pad[..., 1], lpos_pad[..., 2]
        px, py, pz = ppos_pad[..., 0], ppos_pad[..., 1], ppos_pad[..., 2]
        m = {
            # residual inputs carry bout folded in
            "ligx": np.ascontiguousarray(lig_pad.transpose(1, 0, 2) + bout_l[None]),
            "prox": np.ascontiguousarray(
                pro_pad.reshape(SLOTS, NCH, 128, D).transpose(2, 0, 1, 3)
                .reshape(128, SLOTS * NCH, D) + bout_p[None]),
            "ligT": _t_pack(lig_pad, RL),
            "proT": _t_pack(pro_pad, CPC),
            "lposA": np.ascontiguousarray(
                np.stack([-2 * lx, -2 * ly, -2 * lz, np.ones_like(na), na])),
            "pposB": np.ascontiguousarray(
                np.stack([px, py, pz, nb, np.ones_like(nb)])),
            "mpro": np.ascontiguousarray(mpro[None]),                # [1,S,CPC]
            "mligP": np.ascontiguousarray(mligP),                    # [128,S]
        }
        m.update(shared)
        in_maps.append(m)

    meta = dict(NL=NL, NP=NP, CPC=CPC, NCH=NCH, nl=nl, npb=npb,
                lstart=lstart, pstart=pstart, lperm=lperm, pperm=pperm,
                triv_l=triv_l, triv_p=triv_p)
    return in_maps, meta


def _unpack(results, meta):
    NL, NP = meta["NL"], meta["NP"]
    NCH = meta["NCH"]
    nl, npb = meta["nl"], meta["npb"]
    lstart, pstart = meta["lstart"], meta["pstart"]
    lperm, pperm = meta["lperm"], meta["pperm"]
    lig_out = np.zeros((NL, D), np.float32)
    pro_out = np.zeros((NP, D), np.float32)
    for c in range(NCORES):
        ligy = results[c]["ligy"]   # [SLOTS, 128, D]
        proy = results[c]["proy"]   # [SLOTS*NCH, 128, D]
        for s in range(SLOTS):
            b = SLOTS * c + s
            ln, pn = int(nl[b]), int(npb[b])
            if ln > 0:
                lig_out[lperm[lstart[b]:lstart[b + 1]]] = ligy[s, :ln, :]
            for j in range(NCH):
                r0 = j * 128
                n = min(128, pn - r0)
                if n > 0:
                    idx = pperm[pstart[b] + r0: pstart[b] + r0 + n]
                    pro_out[idx] = proy[s * NCH + j, :n, :]
    return lig_out, pro_out


def _numpy_core(m, CPC, NCH, triv_l, triv_p):
    """Numpy mirror of the device program (one core). For validation."""
    def ln(x, g, b):
        mu = x.mean(-1, keepdims=True)
        var = ((x - mu) ** 2).mean(-1, keepdims=True)
        return (x - mu) / np.sqrt(var + EPS) * g + b

    ligy = np.zeros((SLOTS, 128, D), np.float32)
    proy = np.zeros((SLOTS * NCH, 128, D), np.float32)
    ligT = m["ligT"].transpose(1, 0, 2).reshape(D, SLOTS, RL)      # [D,S,RL]
    proT = m["proT"].transpose(1, 0, 2).reshape(D, SLOTS, CPC)
    wt = {k: m[k].transpose(1, 0, 2).reshape(D, D) for k in
          ["wql", "wkp", "wvp", "wqp", "wkl", "wvl", "wol", "wop"]}
    gl, bl, gp, bp = m["gl"][0], m["bl"][0], m["gp"][0], m["bp"][0]
    for s in range(SLOTS):
        lig_s = ligT[:, s, :].T                                    # [RL, D]
        pro_s = proT[:, s, :].T                                    # [CPC, D]
        QT = wt["wql"].T @ ligT[:, s, :] * SCALE                   # [D, RL]
        K2T = wt["wkl"].T @ ligT[:, s, :]
        KT = wt["wkp"].T @ proT[:, s, :]                           # [D, CPC]
        Q2T = wt["wqp"].T @ proT[:, s, :] * SCALE
        V = pro_s @ wt["wvp"]                                      # [CPC, D]
        V2 = lig_s @ wt["wvl"]                                     # [RL, D]
        d2 = (m["lposA"][:, s, :].T @ m["pposB"][:, s, :])         # [RL, CPC]
        bias = np.exp(-np.sqrt(np.maximum(d2, 1e-12)) / 10.0)
        # dir1
        S1 = QT.T @ KT + np.ones((RL, 1), np.float32) @ m["mpro"][:, s, :]
        E = np.exp(S1 + bias)
        rec = 1.0 / E.sum(-1, keepdims=True)
        ctx = E @ V                                                # [RL, D]
        z = (ctx @ wt["wol"]) * rec + m["ligx"][:, s, :]
        ligy[s] = ln(z, gl, bl)
        # dir2 (transposed score layout)
        S2T = K2T.T @ Q2T + m["mligP"][:, s:s + 1]                 # [RL, CPC]
        E2T = np.exp(S2T + bias)
        den2 = E2T.sum(axis=0)                                     # [CPC]
        ctx2T = V2.T @ E2T                                         # [D, CPC]
        z2 = (ctx2T.T @ wt["wop"]) / den2[:, None] + \
            m["prox"][:, s * NCH:(s + 1) * NCH, :].transpose(1, 0, 2).reshape(CPC, D)
        z2 = ln(z2, gp, bp)
        for j in range(NCH):
            proy[s * NCH + j] = z2[j * 128:(j + 1) * 128]
    return {"ligy": ligy, "proy": proy}


def _build_program(CPC, triv_l, triv_p):
    import concourse.mybir as mybir
    import concourse.tile as tile
    from concourse import bacc
    from concourse.masks import make_identity

    NCH = CPC // 128
    f32 = mybir.dt.float32
    f32r = mybir.dt.float32r
    AF = mybir.ActivationFunctionType
    OP = mybir.AluOpType

    nc = bacc.Bacc("TRN2", target_bir_lowering=False, debug=False,
                   num_devices=NCORES)

    # f32r DRAM tensors carry plain fp32 bits; declaring them f32r lets
    # plain DMAs land in f32r SBUF tiles with no cast.
    din = {}
    for name, shape, dt_ in [
        ("ligx", [128, SLOTS, D], f32), ("prox", [128, SLOTS * NCH, D], f32),
        ("ligT", [128, 2, SLOTS * RL], f32r), ("proT", [128, 2, SLOTS * CPC], f32r),
        ("lposA", [5, SLOTS, RL], f32r), ("pposB", [5, SLOTS, CPC], f32r),
        ("mpro", [1, SLOTS, CPC], f32r), ("mligP", [128, SLOTS], f32),
        ("onesr", [1, 128], f32r), ("onesc", [128, 1], f32r),
        ("wql", [128, 2, D], f32r), ("wkp", [128, 2, D], f32r),
        ("wvp", [128, 2, D], f32r), ("wqp", [128, 2, D], f32r),
        ("wkl", [128, 2, D], f32r), ("wvl", [128, 2, D], f32r),
        ("wol", [128, 2, D], f32r), ("wop", [128, 2, D], f32r),
        ("gl", [128, D], f32), ("bl", [128, D], f32),
        ("gp", [128, D], f32), ("bp", [128, D], f32),
    ]:
        din[name] = nc.dram_tensor(name, shape, dt_, kind="ExternalInput")
    ligy_d = nc.dram_tensor("ligy", [SLOTS, 128, D], f32, kind="ExternalOutput")
    proy_d = nc.dram_tensor("proy", [SLOTS * NCH, 128, D], f32,
                            kind="ExternalOutput")

    # 512/128 col splits (psum-bank aligned)
    splits = [(n0, min(n0 + 512, CPC)) for n0 in range(0, CPC, 512)]

    with tile.TileContext(nc) as tc:
        with tc.tile_pool(name="const", bufs=1) as cp, \
             tc.tile_pool(name="slotin", bufs=2) as sip, \
             tc.tile_pool(name="slotp", bufs=2) as spp, \
             tc.tile_pool(name="wk3", bufs=3) as wk3, \
             tc.tile_pool(name="wk2", bufs=2) as wk2, \
             tc.tile_pool(name="stat", bufs=16) as stp, \
             tc.tile_pool(name="psA", bufs=3, space="PSUM") as psA, \
             tc.tile_pool(name="psB", bufs=2, space="PSUM") as psB:

            def load(name, dt_=None, eng=None):
                t = cp.tile(din[name].shape, dt_ or f32, tag=name)
                (eng or nc.sync).dma_start(t[:], din[name].ap()[:])
                return t

            # upfront loads (lig side + weights + consts)
            ligx = load("ligx")
            ligT = load("ligT", f32r, nc.gpsimd)
            lposA = load("lposA", f32r)
            mligP = load("mligP")
            onesr = load("onesr", f32r)
            onesc = load("onesc", f32r)
            wql = load("wql", f32r, nc.scalar)
            wkp = load("wkp", f32r, nc.scalar)
            wvp = load("wvp", f32r, nc.scalar)
            wqp = load("wqp", f32r, nc.scalar)
            wkl = load("wkl", f32r, nc.scalar)
            wvl = load("wvl", f32r, nc.scalar)
            wol = load("wol", f32r, nc.scalar)
            wop = load("wop", f32r, nc.scalar)
            gl = load("gl") if not triv_l else None
            bl = load("bl") if not triv_l else None
            gp = load("gp") if not triv_p else None
            bp = load("bp") if not triv_p else None

            ident = cp.tile([128, 128], f32, tag="ident")
            make_identity(nc, ident[:])
            epsb = cp.tile([128, 1], f32, tag="epsb")
            nc.vector.memset(epsb[:], EPS)

            # lig-side projections (small, upfront); all-slot width
            QT = cp.tile([128, 2, SLOTS * RL], f32r, tag="QT")
            K2T = cp.tile([128, 2, SLOTS * RL], f32r, tag="K2T")
            V2 = cp.tile([128, SLOTS, D], f32r, tag="V2")
            for g in range(2):
                ps = psA.tile([128, CPC], f32, tag="big")
                for h in range(2):
                    nc.tensor.matmul(ps[:, :SLOTS * RL], wql[:, h, 128 * g:128 * (g + 1)],
                                     ligT[:, h, :], start=(h == 0), stop=(h == 1))
                nc.scalar.activation(QT[:, g, :], ps[:, :SLOTS * RL], AF.Copy,
                                     scale=SCALE)
                ps2 = psA.tile([128, CPC], f32, tag="big")
                for h in range(2):
                    nc.tensor.matmul(ps2[:, :SLOTS * RL], wkl[:, h, 128 * g:128 * (g + 1)],
                                     ligT[:, h, :], start=(h == 0), stop=(h == 1))
                nc.vector.tensor_copy(K2T[:, g, :], ps2[:, :SLOTS * RL])
            for k in range(SLOTS):
                ps = psB.tile([128, D], f32, tag="small")
                for h in range(2):
                    nc.tensor.matmul(ps[:], ligT[:, h, 128 * k:128 * (k + 1)],
                                     wvl[:, h, :], start=(h == 0), stop=(h == 1))
                nc.vector.tensor_copy(V2[:, k, :], ps[:])

            def epilogue(zp, rec_ap, x_ap, g, b, out_ap):
                w = wk3.tile([128, D], f32, tag="w256")
                msum = stp.tile([128, 1], f32, tag="stat")
                nc.vector.scalar_tensor_tensor(
                    w[:], zp, rec_ap, x_ap, op0=OP.mult, op1=OP.add,
                    accum_out=msum[:])
                negmu = stp.tile([128, 1], f32, tag="stat")
                nc.vector.tensor_scalar_mul(negmu[:], msum[:], -1.0 / D)
                wc = wk3.tile([128, D], f32, tag="w256")
                nc.scalar.activation(wc[:], w[:], AF.Identity, bias=negmu[:])
                sq = wk3.tile([128, D], f32, tag="w256")
                ssq = stp.tile([128, 1], f32, tag="stat")
                nc.vector.scalar_tensor_tensor(
                    sq[:], wc[:], 1.0, wc[:], op0=OP.mult, op1=OP.mult,
                    accum_out=ssq[:])
                stdt = stp.tile([128, 1], f32, tag="stat")
                nc.scalar.activation(stdt[:], ssq[:], AF.Sqrt, scale=1.0 / D,
                                     bias=epsb[:])
                rstd = stp.tile([128, 1], f32, tag="stat")
                nc.vector.reciprocal(rstd[:], stdt[:])
                o = wk3.tile([128, D], f32, tag="w256")
                if g is None:
                    nc.scalar.activation(o[:], wc[:], AF.Identity, scale=rstd[:])
                else:
                    nc.vector.scalar_tensor_tensor(
                        o[:], wc[:], rstd[:], g[:], op0=OP.mult, op1=OP.mult)
                    nc.vector.tensor_tensor(o[:], o[:], b[:], OP.add)
                nc.sync.dma_start(out_ap, o[:])

            dma_engs = [nc.sync, nc.scalar, nc.gpsimd, nc.sync]
            for s in range(SLOTS):
                # ---- per-slot pro-side loads ----
                proT_s = sip.tile([128, 2, CPC], f32r, tag="proT")
                dma_engs[s % 4].dma_start(
                    proT_s[:], din["proT"].ap()[:, :, CPC * s:CPC * (s + 1)])
                prox_s = sip.tile([128, NCH, D], f32, tag="prox")
                dma_engs[(s + 1) % 4].dma_start(
                    prox_s[:], din["prox"].ap()[:, NCH * s:NCH * (s + 1), :])
                pposB_s = sip.tile([5, CPC], f32r, tag="pposB")
                nc.sync.dma_start(pposB_s[:], din["pposB"].ap()[:, s, :])
                mpro_s = sip.tile([1, CPC], f32r, tag="mpro")
                nc.sync.dma_start(mpro_s[:], din["mpro"].ap()[:, s, :])

                # ---- per-slot pro-side projections ----
                KT_s = spp.tile([128, 2, CPC], f32r, tag="KT")
                Q2T_s = spp.tile([128, 2, CPC], f32r, tag="Q2T")
                for g in range(2):
                    ps = psA.tile([128, CPC], f32, tag="big")
                    for n0, n1 in splits:
                        for h in range(2):
                            nc.tensor.matmul(
                                ps[:, n0:n1], wkp[:, h, 128 * g:128 * (g + 1)],
                                proT_s[:, h, n0:n1], start=(h == 0), stop=(h == 1))
                    nc.vector.tensor_copy(KT_s[:, g, :], ps[:])
                    ps2 = psA.tile([128, CPC], f32, tag="big")
                    for n0, n1 in splits:
                        for h in range(2):
                            nc.tensor.matmul(
                                ps2[:, n0:n1], wqp[:, h, 128 * g:128 * (g + 1)],
                                proT_s[:, h, n0:n1], start=(h == 0), stop=(h == 1))
                    nc.scalar.activation(Q2T_s[:, g, :], ps2[:], AF.Copy,
                                         scale=SCALE)
                V_s = spp.tile([128, NCH, D], f32r, tag="V")
                for k in range(NCH):
                    ps = psB.tile([128, D], f32, tag="small")
                    for h in range(2):
                        nc.tensor.matmul(ps[:], proT_s[:, h, 128 * k:128 * (k + 1)],
                                         wvp[:, h, :], start=(h == 0), stop=(h == 1))
                    nc.vector.tensor_copy(V_s[:, k, :], ps[:])

                # ---- shared distance bias ----
                d2p = psA.tile([128, CPC], f32, tag="big")
                for n0, n1 in splits:
                    nc.tensor.matmul(d2p[:, n0:n1], lposA[:, s, :],
                                     pposB_s[:, n0:n1], start=True, stop=True)
                d2c = wk3.tile([128, CPC], f32, tag="w640")
                nc.vector.tensor_scalar_max(d2c[:], d2p[:], 1e-12)
                dist = wk3.tile([128, CPC], f32, tag="w640")
                nc.scalar.activation(dist[:], d2c[:], AF.Sqrt)
                bias = wk2.tile([128, CPC], f32, tag="bias")
                nc.scalar.activation(bias[:], dist[:], AF.Exp, scale=-0.1)

                # ---------------- dir-1: lig rows <- pro cols ----------------
                sp = psA.tile([128, CPC], f32, tag="big")
                for n0, n1 in splits:
                    nc.tensor.matmul(sp[:, n0:n1], QT[:, 0, RL * s:RL * (s + 1)],
                                     KT_s[:, 0, n0:n1], start=True, stop=False)
                    nc.tensor.matmul(sp[:, n0:n1], QT[:, 1, RL * s:RL * (s + 1)],
                                     KT_s[:, 1, n0:n1], start=False, stop=False)
                    nc.tensor.matmul(sp[:, n0:n1], onesr[:],
                                     mpro_s[:, n0:n1], start=False, stop=True)
                ein = wk3.tile([128, CPC], f32, tag="w640")
                nc.vector.tensor_tensor(ein[:], sp[:], bias[:], OP.add)
                e1 = wk3.tile([128, CPC], f32, tag="w640")
                den = stp.tile([128, 1], f32, tag="stat")
                nc.scalar.activation(e1[:], ein[:], AF.Exp, accum_out=den[:])
                rec = stp.tile([128, 1], f32, tag="stat")
                nc.vector.reciprocal(rec[:], den[:])

                at = wk3.tile([128, NCH, 128], f32r, tag="at")
                for j in range(NCH):
                    tp = psB.tile([128, 128], f32, tag="small")
                    nc.tensor.transpose(tp[:], e1[:, 128 * j:128 * (j + 1)],
                                        ident[:])
                    nc.vector.tensor_copy(at[:, j, :], tp[:])
                ctxp = psB.tile([128, D], f32, tag="small")
                for j in range(NCH):
                    nc.tensor.matmul(ctxp[:], at[:, j, :], V_s[:, j, :],
                                     start=(j == 0), stop=(j == NCH - 1))
                ctxs = wk2.tile([128, D], f32, tag="ctx")
                nc.scalar.activation(ctxs[:], ctxp[:], AF.Copy)
                ctxT = wk2.tile([128, 2, 128], f32r, tag="ctxT")
                for h in range(2):
                    tp = psB.tile([128, 128], f32, tag="small")
                    nc.tensor.transpose(tp[:], ctxs[:, 128 * h:128 * (h + 1)],
                                        ident[:])
                    nc.vector.tensor_copy(ctxT[:, h, :], tp[:])
                zp = psB.tile([128, D], f32, tag="small")
                for h in range(2):
                    nc.tensor.matmul(zp[:], ctxT[:, h, :], wol[:, h, :],
                                     start=(h == 0), stop=(h == 1))
                epilogue(zp[:], rec[:], ligx[:, s, :], gl, bl, ligy_d.ap()[s])

                # ------- dir-2 (transposed scores): pro rows <- lig cols -------
                s2p = psA.tile([128, CPC], f32, tag="big")
                for n0, n1 in splits:
                    nc.tensor.matmul(s2p[:, n0:n1], K2T[:, 0, RL * s:RL * (s + 1)],
                                     Q2T_s[:, 0, n0:n1], start=True, stop=False)
                    nc.tensor.matmul(s2p[:, n0:n1], K2T[:, 1, RL * s:RL * (s + 1)],
                                     Q2T_s[:, 1, n0:n1], start=False, stop=True)
                ein2 = wk3.tile([128, CPC], f32, tag="w640")
                nc.vector.tensor_tensor(ein2[:], s2p[:], bias[:], OP.add)
                e2t = wk3.tile([128, CPC], f32r, tag="e2t")
                nc.scalar.activation(e2t[:], ein2[:], AF.Exp,
                                     bias=mligP[:, s:s + 1])
                # denominators: column sums via ones matmul, then transpose
                dens = wk2.tile([1, CPC], f32, tag="dens")
                for n0, n1 in splits:
                    dp = psB.tile([1, 512], f32, tag="small")
                    nc.tensor.matmul(dp[:, :n1 - n0], onesc[:], e2t[:, n0:n1],
                                     start=True, stop=True)
                    nc.vector.tensor_copy(dens[:, n0:n1], dp[:, :n1 - n0])
                den2 = stp.tile([128, NCH], f32, tag="statN")
                for j in range(NCH):
                    dtp = psB.tile([128, 1], f32, tag="small")
                    nc.tensor.matmul(dtp[:], dens[:, 128 * j:128 * (j + 1)],
                                     ident[0:1, 0:1], start=True, stop=True)
                    nc.vector.tensor_copy(den2[:, j:j + 1], dtp[:])
                rec2 = stp.tile([128, NCH], f32, tag="statN")
                nc.vector.reciprocal(rec2[:], den2[:])

                ctx2T = wk2.tile([128, 2, CPC], f32r, tag="c2s")
                for h in range(2):
                    cp2 = psA.tile([128, CPC], f32, tag="big")
                    for n0, n1 in splits:
                        nc.tensor.matmul(cp2[:, n0:n1], V2[:, s, 128 * h:128 * (h + 1)],
                                         e2t[:, n0:n1], start=True, stop=True)
                    nc.scalar.activation(ctx2T[:, h, :], cp2[:], AF.Copy)
                for j in range(NCH):
                    zp2 = psB.tile([128, D], f32, tag="small")
                    for h in range(2):
                        nc.tensor.matmul(zp2[:], ctx2T[:, h, 128 * j:128 * (j + 1)],
                                         wop[:, h, :], start=(h == 0), stop=(h == 1))
                    epilogue(zp2[:], rec2[:, j:j + 1], prox_s[:, j, :],
                             gp, bp, proy_d.ap()[NCH * s + j])

    nc.compile()
    return nc


def _ensure_ntff_hook():
    """Register the axon NTFF profiling hook if the image lacks
    antenv.axon_hooks (bass_utils imports it when trace=True)."""
    try:
        from antenv.axon_hooks import get_axon_ntff_profile_hook  # noqa: F401
        return
    except ImportError:
        pass
    import types
    import antenv
    mod = types.ModuleType("antenv.axon_hooks")
    state = {"h": None}
    mod.set_axon_ntff_profile_hook = lambda h: state.__setitem__("h", h)
    mod.get_axon_ntff_profile_hook = lambda: state["h"]
    sys.modules["antenv.axon_hooks"] = mod
    antenv.axon_hooks = mod
    try:
        from trn_agent_boot.trn_boot import _ntff_profile_via_ctypes
        mod.set_axon_ntff_profile_hook(
            _ntff_profile_via_ctypes("/opt/axon/libaxon_pjrt.so"))
    except Exception:
        pass


def _run_device(in_maps, meta, trace=False):
    if trace:
        _ensure_ntff_hook()
    from concourse.bass_utils import run_bass_kernel_spmd
    key = (meta["CPC"], meta["triv_l"], meta["triv_p"])
    if key not in _PROG_CACHE:
        _PROG_CACHE[key] = _build_program(*key)
    nc = _PROG_CACHE[key]
    res = run_bass_kernel_spmd(nc, in_maps, core_ids=list(range(NCORES)),
                               trace=trace)
    return res


def kernel(**inputs):
    in_maps, meta = _prepare(inputs)
    if os.environ.get("KERNEL_NUMPY"):
        results = [_numpy_core(m, meta["CPC"], meta["NCH"],
                               meta["triv_l"], meta["triv_p"])
                   for m in in_maps]
    else:
        results = _run_device(in_maps, meta).results
    return _unpack(results, meta)


def kernel_traced(**inputs):
    """Like kernel() but returns (outputs, BassKernelResults) with profiling."""
    in_maps, meta = _prepare(inputs)
    res = _run_device(in_maps, meta, trace=True)
    return _unpack(res.results, meta), res
